# revision 75
# baseline (speedup 1.0000x reference)
"""Trainium2 Bass kernel for nn_LocalNeighborhood (retrieval_knn).

Problem: first_index [B=4, L=4096, 1] int64 (sorted along L), attr [B, L, D=128] f32.
reference: K=16 nearest neighbors per query by |center_i - center_j| (stable argsort
tie-break by index), gather attr rows -> [B, L, 16, 128] f32.

Because centers are sorted along L, each query's 16 nearest neighbors live in the
index window [i-15, i+15]. Per-query merge ranks are computed with vector-engine
equality counting (exact), yielding a window position pos[r] in [0,31) per output
slot.

Gather strategy (v2): PE one-hot matmul instead of SWDGE dma_gather (the old
dma_gather serialized ~98us of descriptor generation on the GpSimd engine).
Key fact: the 32 queries of an aligned 32-query span [32m, 32m+32) only reference
attr rows [32m-15, 32m+46] -- 62 rows inside the 128-row slab block starting at
row 32m-15. So for each m: one 128x512 matmul
    out[d, slot] = slab_m[src, d]^T @ onehot[src, slot]
gathers all 512 output rows (32 queries x 16 neighbors) of the span, where
onehot[src, slot] = [src == idxrel(slot)] is built by one DVE is_equal against a
per-partition iota, and idxrel = (q mod 32) + pos in [0, 62) is the block-local
row of slot's neighbor.

To get idxrel replicated across all 128 partitions with slot on the free axis
(required for the is_equal), idxrel is round-tripped through a DRAM scratch
buffer: one DMA stages it slot-linear, a second DMA reads it back with a
step-0 partition dim (partition-broadcast read).

Output is written transposed ([D, HALF*K] bf16 per core); the host transposes
and upcasts. Sharding: 8 cores = (batch b = core//2) x (half of L).
"""

import numpy as np

B, L, D, K = 4, 4096, 128, 16
NCORES = 8
HALF = L // 2              # 2048 queries per core
P = 128                    # partitions
G = HALF // P              # 16 query-groups (q = 128*g + p)
W = 31                     # candidate window size per query [i-15, i+15]
FPAD = 16                  # attr row padding in front
BPAD = 112                 # attr row padding in back (block 63 reads to +2143)
ROWS2 = B * L + FPAD + BPAD
NBLK = 64                  # slab blocks per core (block m: rows [32m, 32m+128))
SLABROWS = 32 * (NBLK - 1) + P   # 2144 per-core slab rows
SLICES = (2, 2, 4, 4, 4)   # groups per pipeline slice (sum = G)
MM_N = 512                 # moving cols per matmul (4 tiles = 32 queries * 16)
BIG = np.float32(1e9)

_CACHE = {}


def _view(ap, offset, dims):
    """AP over the same tensor: keep ap's partition dim, custom free dims.

    dims: list of (step_elems, num). offset in elements (within a partition).
    """
    from concourse.bass import AP
    part = list(ap.ap[0])
    return AP(ap.tensor, ap.offset + offset, [part] + [list(d) for d in dims])


def _dview(handle, offset, dims):
    """Free-form AP over a DRAM tensor. dims: list of (step_elems, num);
    first dim pairs with SBUF partitions."""
    from concourse.bass import AP
    ap = handle[:]
    return AP(ap.tensor, ap.offset + offset, [list(d) for d in dims])


def _emit(tc, nc, io):
    import concourse.mybir as mybir
    from concourse import bass, tile  # noqa: F401
    from concourse.mybir import AluOpType as op, AxisListType as ax

    f32 = mybir.dt.float32
    bf16 = mybir.dt.bfloat16

    (ctr_d, cadd_d, iota16_d, c16m_d, g128_d, iotap_d, attr_d, idxs_d, out_d) = io

    import contextlib
    with contextlib.ExitStack() as ctx:
        cpool = ctx.enter_context(tc.tile_pool(name="consts", bufs=1))
        wpool = ctx.enter_context(tc.tile_pool(name="work", bufs=1))
        spool = ctx.enter_context(tc.tile_pool(name="scratch", bufs=1))
        bpool = ctx.enter_context(tc.tile_pool(name="idxb", bufs=2))
        rpool = ctx.enter_context(tc.tile_pool(name="rbcast", bufs=2))
        r0pool = ctx.enter_context(tc.tile_pool(name="rbcast0", bufs=1))
        opool = ctx.enter_context(tc.tile_pool(name="onehot", bufs=3))
        ppool = ctx.enter_context(tc.tile_pool(name="psum", bufs=3, space="PSUM"))
        fpool = ctx.enter_context(tc.tile_pool(name="outstg", bufs=3))

        def load(pool, src, shape, dtype=f32):
            t = pool.tile(shape, dtype, name=f"ld_{src.name}")
            nc.sync.dma_start(out=t[:], in_=src[:])
            return t

        # split ctr load: slices 0-1 (cols [0, 62)) arrive first so the DVE
        # pipeline starts early; the rest streams in behind it.
        ctr = cpool.tile([P, G * W], f32, name="ld_ctr_win")
        nc.sync.dma_start(out=ctr[:, :62], in_=ctr_d[:][:, :62])
        iota16 = load(cpool, iota16_d, [P, 16])
        i31 = load(cpool, g128_d, [P, 31])
        i31d64 = load(cpool, c16m_d, [P, 31])
        cadd = load(cpool, cadd_d, [P, 1])
        iotap = load(cpool, iotap_d, [P, 1])
        nc.sync.dma_start(out=ctr[:, 62:], in_=ctr_d[:][:, 62:])
        # bf16 casts of the small consts are emitted inside slice 0 (after
        # the first plane op) so they don't block the DVE pipeline start.
        iota16b = cpool.tile([P, 16], bf16, name="iota16b")
        # i31c[p, w] = w + cadd[p]: folds the block-local window base into
        # the position weights, so the pos-reduce emits idxrel directly
        # (sum_w EQ16[r,w] == 1 exactly, so the cadd term survives once).
        i31c = cpool.tile([P, 31], bf16, name="i31c")

        # attr slab: slab[p, 128*m + d] = attr2[core0 + 32*m + p, d] (bf16)
        # loaded in 4 chunks of 16 blocks each for DMA-queue parallelism.
        slab = cpool.tile([P, NBLK * D], bf16, name="slab")
        for h in range(4):
            nc.sync.dma_start(
                out=slab[:, h * 16 * D:(h + 1) * 16 * D],
                in_=_dview(attr_d, (h * 16 * 32) * D,
                           [(D, P), (32 * D, 16), (1, D)]))

        def tt(o, a, b, alu):
            nc.vector.tensor_tensor(out=o, in0=a, in1=b, op=alu)

        def red(o, a, alu=op.add):
            nc.vector.tensor_reduce(out=o, in_=a, axis=ax.X, op=alu)

        _wcnt = [0]

        g0 = 0
        pending = None
        for sl, NG in enumerate(SLICES):

            # ---- rank pipeline (DVE): pos[r] in [0,31) per (q, r) ----
            def wtile(n):
                _wcnt[0] += 1
                return wpool.tile([P, n], f32, name=f"w{_wcnt[0]}")

            cof = g0 * W

            # key[w] = |c_q - c_w| + w/64 : exact f32 (dist<=1e5 int, 17+6
            # bits < 24), unique per window, orders exactly by (dist, index).
            diff = wtile(31 * NG)
            tt(diff, _view(ctr, cof + 15, [(W, NG), (0, 31)]),
                     _view(ctr, cof + 0, [(W, NG), (1, 31)]), op.subtract)
            absd = wtile(31 * NG)
            nc.vector.scalar_tensor_tensor(
                out=absd, in0=diff, scalar=-1.0, in1=diff, op0=op.mult,
                op1=op.max)
            key = wtile(31 * NG)
            tt(key, absd, _view(i31d64, 0, [(0, NG), (1, 31)]), op.add)
            # rank[w] = #{w' : key(w') < key(w)} via one [w, w'] plane
            RK = spool.tile([P, 961 * NG], bf16, name=f"rkpl{sl}", tag=f"plane{sl}")
            tt(RK, _view(key, 0, [(31, NG), (0, 31), (1, 31)]),
                   _view(key, 0, [(31, NG), (1, 31), (0, 31)]), op.is_lt)
            if sl == 0:
                nc.vector.tensor_copy(out=iota16b, in_=iota16)
                nc.vector.tensor_scalar(out=i31c, in0=i31,
                                        scalar1=cadd[:, 0:1], scalar2=None,
                                        op0=op.add)
            rank = wpool.tile([P, 31 * NG], bf16, name=f"rank{sl}", tag=f"rankw{sl}")
            with nc.allow_low_precision(reason="0/1 plane sums <= 31, exact in bf16"):
                red(rank, _view(RK, 0, [(961, NG), (31, 31), (1, 31)]))
            # pos[r] = sum_w [rank(w) == r] * w
            EQ16 = spool.tile([P, 496 * NG], bf16, name=f"eq16{sl}", tag=f"plane2_{sl}")
            tt(EQ16, _view(iota16b, 0, [(0, NG), (1, 16), (0, 31)]),
                     _view(rank, 0, [(31, NG), (0, 16), (1, 31)]), op.is_equal)
            POSP = spool.tile([P, 496 * NG], bf16, name=f"posp{sl}", tag=f"plane3_{sl}")
            tt(POSP, EQ16, _view(i31c, 0, [(0, NG), (0, 16), (1, 31)]), op.mult)
            # block-local neighbor row: idxrel = (q mod 32) + pos in [0, 62)
            idxb = bpool.tile([P, 16 * NG], bf16, name=f"idxb{sl}", tag="idxb")
            with nc.allow_low_precision(reason="one-hot dot, values <= 61 exact in bf16"):
                red(idxb[:], _view(POSP, 0, [(496, NG), (31, 16), (1, 31)]))

            # ---- DRAM round-trip: slot-linear stage, partition-bcast read ----
            # slot = q*16 + r = 2048*g + 16*p + r
            nc.sync.dma_start(
                out=_dview(idxs_d, 2048 * g0, [(16, P), (2048, NG), (1, 16)]),
                in_=idxb[:])
            rp = r0pool if sl == 0 else rpool
            R = rp.tile([P, 2048 * NG], bf16, name=f"R{sl}",
                        tag="R0" if sl == 0 else "R")
            for bi in range(NG):
                nc.sync.dma_start(
                    out=R[:, 2048 * bi:2048 * (bi + 1)],
                    in_=_dview(idxs_d, 2048 * (g0 + bi), [(0, P), (1, 2048)]))

            # ---- per 32-query span: onehot (DVE, 4 spans per op), matmul
            # (PE), PSUM-bank-pair copy (ACT) + store.  Emission of slice
            # k's gather work is delayed until after slice k+1's rank ops:
            # the DVE queue is in-order, so an onehot waiting on its R
            # broadcast DMA would head-of-line block the next slice's ranks.
            def gather_work(sl, g0, NG, R, bis):
                for bi in bis:
                    oh = opool.tile([P, 4 * MM_N], bf16, name=f"oh{sl}_{bi}",
                                    tag="oh")
                    nc.vector.tensor_scalar(
                        out=oh, in0=R[:, 4 * MM_N * bi:4 * MM_N * (bi + 1)],
                        scalar1=iotap[:, 0:1], scalar2=None, op0=op.is_equal)
                    gf = fpool.tile([P, 4 * MM_N], bf16, name=f"gf{sl}_{bi}",
                                    tag="gf")
                    for hf in range(2):
                        ps = ppool.tile([P, 2 * MM_N], f32,
                                        name=f"ps{sl}_{bi}_{hf}", tag="ps")
                        for k in range(2):
                            m = 4 * (g0 + bi) + 2 * hf + k
                            nc.tensor.matmul(
                                ps[:, MM_N * k:MM_N * (k + 1)],
                                slab[:, D * m:D * (m + 1)],
                                oh[:, MM_N * (2 * hf + k):
                                   MM_N * (2 * hf + k + 1)],
                                start=True, stop=True)
                        nc.scalar.copy(
                            out=gf[:, 2 * MM_N * hf:2 * MM_N * (hf + 1)],
                            in_=ps)
                    m0 = 4 * (g0 + bi)
                    # the reserved batch (emitted last) drains in 2 stores
                    last = (sl == 0 and bi == SLICES[0] - 1)
                    for st in range(2 if last else 1):
                        w2 = 2 * MM_N
                        o = st * w2
                        nc.scalar.dma_start(
                            out=_dview(out_d, MM_N * m0 + (o if last else 0),
                                       [(HALF * K, P),
                                        (1, w2 if last else 4 * MM_N)]),
                            in_=gf[:, o:o + w2] if last else gf[:])

            # delayed emission: slice k's gathers run under slice k+1's
            # ranks.  Slice 0's final batch is RESERVED for the very end:
            # its R has been resident for the whole kernel, so the kernel
            # closes with a zero-R-latency gather chain instead of waiting
            # on the last slice's DRAM round-trip.
            if pending is not None:
                psl, pg0, pNG, pR = pending
                bis = list(range(pNG - 1)) if psl == 0 else list(range(pNG))
                gather_work(psl, pg0, pNG, pR, bis)
            pending = (sl, g0, NG, R)
            if sl == 0:
                work0 = pending
            g0 += NG
        gather_work(*pending, list(range(pending[2])))
        gather_work(*work0, [SLICES[0] - 1])


def build():
    """Build + compile the SPMD program once. Returns the Bacc."""
    if "prog" in _CACHE:
        return _CACHE["prog"]
    from concourse import bacc, tile
    import concourse.mybir as mybir

    f32 = mybir.dt.float32
    bf16 = mybir.dt.bfloat16
    nc = bacc.Bacc("TRN2", target_bir_lowering=False, debug=False,
                   num_devices=NCORES)
    ctr_d = nc.declare_dram_parameter("ctr_win", [P, G * W], f32, isOutput=False)
    cadd_d = nc.declare_dram_parameter("c_add", [P, 1], f32, isOutput=False)
    iota16_d = nc.declare_dram_parameter("c_iota16", [P, 16], f32, isOutput=False)
    c16m_d = nc.declare_dram_parameter("c_i31d64", [P, 31], f32, isOutput=False)
    g128_d = nc.declare_dram_parameter("c_i31", [P, 31], f32, isOutput=False)
    iotap_d = nc.declare_dram_parameter("c_iotap", [P, 1], f32, isOutput=False)
    attr_d = nc.declare_dram_parameter("attr_slab", [SLABROWS, D], bf16,
                                       isOutput=False)
    idxs_d = nc.dram_tensor("idx_scratch", [HALF * K], bf16)
    out_d = nc.declare_dram_parameter("out", [D, HALF * K], bf16, isOutput=True)

    io = (ctr_d, cadd_d, iota16_d, c16m_d, g128_d, iotap_d, attr_d, idxs_d,
          out_d)
    with tile.TileContext(nc) as tc:
        _emit(tc, nc, io)
    nc.compile()
    _CACHE["prog"] = nc
    return nc


def host_inputs(first_index, attr):
    """Shard + pad on the host. Returns in_maps (one dict per core)."""
    center = np.asarray(first_index)[..., 0].astype(np.float32)  # [B, L]
    attr = np.ascontiguousarray(np.asarray(attr), dtype=np.float32)

    import ml_dtypes
    attr_bf16 = np.zeros((ROWS2, D), ml_dtypes.bfloat16)
    attr_bf16[FPAD:FPAD + B * L] = attr.reshape(B * L, D).astype(ml_dtypes.bfloat16)
    # per-core slab: local row l <-> original flat row b*L + r0 + l - 15

    LPAD = L + 32
    cpad = np.empty((B, LPAD), np.float32)
    cpad[:, :16] = -BIG
    cpad[:, 16:16 + L] = center
    cpad[:, 16 + L:] = BIG

    p = np.arange(P)
    gg = np.arange(G)
    t = np.arange(W)
    iota16 = np.broadcast_to(np.arange(16, dtype=np.float32), (P, 16)).copy()
    i31 = np.arange(31, dtype=np.float32)
    consts = {
        "c_add": (p % 32).astype(np.float32)[:, None],
        "c_iota16": iota16,
        "c_i31d64": np.broadcast_to(i31 / 64.0, (P, 31)).copy(),
        "c_i31": np.broadcast_to(i31, (P, 31)).copy(),
        "c_iotap": p.astype(np.float32)[:, None],
    }

    in_maps = []
    for c in range(NCORES):
        b, h = divmod(c, 2)
        r0 = h * HALF
        # ctr_win[p, g*31 + t] = cpad[b, r0 + g*128 + p + t + 1]
        idx = r0 + gg[None, :, None] * P + p[:, None, None] + t[None, None, :] + 1
        ctr_win = cpad[b][idx].reshape(P, G * W).astype(np.float32)
        m = dict(consts)
        m["ctr_win"] = np.ascontiguousarray(ctr_win)
        s0 = b * L + r0 + 1
        m["attr_slab"] = np.ascontiguousarray(attr_bf16[s0:s0 + SLABROWS])
        in_maps.append(m)
    return in_maps


def kernel(first_index, attr):
    from concourse.bass_utils import run_bass_kernel_spmd

    nc = build()
    in_maps = host_inputs(first_index, attr)
    res = run_bass_kernel_spmd(nc, in_maps, list(range(NCORES)))
    out = np.empty((B, L, K, D), np.float32)
    for c in range(NCORES):
        b, h = divmod(c, 2)
        r0 = h * HALF
        o = np.asarray(res.results[c]["out"], dtype=np.float32)  # [D, HALF*K]
        out[b, r0:r0 + HALF] = o.T.reshape(HALF, K, D)
    return out


# revision 76
# speedup vs baseline: 1.1533x; 1.1533x over previous
"""Trainium2 Bass kernel for nn_LocalNeighborhood (retrieval_knn).

Problem: first_index [B=4, L=4096, 1] int64 (sorted along L), attr [B, L, D=128] f32.
reference: K=16 nearest neighbors per query by |center_i - center_j| (stable argsort
tie-break by index), gather attr rows -> [B, L, 16, 128] f32.

Because centers are sorted along L, each query's 16 nearest neighbors live in the
index window [i-15, i+15]. Per-query merge ranks are computed with vector-engine
equality counting (exact), yielding a window position pos[r] in [0,31) per output
slot.

Gather strategy (v2): PE one-hot matmul instead of SWDGE dma_gather (the old
dma_gather serialized ~98us of descriptor generation on the GpSimd engine).
Key fact: the 32 queries of an aligned 32-query span [32m, 32m+32) only reference
attr rows [32m-15, 32m+46] -- 62 rows inside the 128-row slab block starting at
row 32m-15. So for each m: one 128x512 matmul
    out[d, slot] = slab_m[src, d]^T @ onehot[src, slot]
gathers all 512 output rows (32 queries x 16 neighbors) of the span, where
onehot[src, slot] = [src == idxrel(slot)] is built by one DVE is_equal against a
per-partition iota, and idxrel = (q mod 32) + pos in [0, 62) is the block-local
row of slot's neighbor.

To get idxrel replicated across all 128 partitions with slot on the free axis
(required for the is_equal), idxrel is round-tripped through a DRAM scratch
buffer: one DMA stages it slot-linear, a second DMA reads it back with a
step-0 partition dim (partition-broadcast read).

Output is written transposed ([D, HALF*K] bf16 per core); the host transposes
and upcasts. Sharding: 8 cores = (batch b = core//2) x (half of L).
"""

import numpy as np

B, L, D, K = 4, 4096, 128, 16
NCORES = 8
HALF = L // 2              # 2048 queries per core
P = 128                    # partitions
G = HALF // P              # 16 query-groups (q = 128*g + p)
W = 31                     # candidate window size per query [i-15, i+15]
FPAD = 16                  # attr row padding in front
BPAD = 112                 # attr row padding in back (block 63 reads to +2143)
ROWS2 = B * L + FPAD + BPAD
NBLK = 64                  # slab blocks per core (block m: rows [32m, 32m+128))
SLABROWS = 32 * (NBLK - 1) + P   # 2144 per-core slab rows
SLICES = (2, 2, 4, 4, 4)   # groups per pipeline slice (sum = G)
MM_N = 512                 # moving cols per matmul (4 tiles = 32 queries * 16)
BIG = np.float32(1e9)

_CACHE = {}


def _view(ap, offset, dims):
    """AP over the same tensor: keep ap's partition dim, custom free dims.

    dims: list of (step_elems, num). offset in elements (within a partition).
    """
    from concourse.bass import AP
    part = list(ap.ap[0])
    return AP(ap.tensor, ap.offset + offset, [part] + [list(d) for d in dims])


def _dview(handle, offset, dims):
    """Free-form AP over a DRAM tensor. dims: list of (step_elems, num);
    first dim pairs with SBUF partitions."""
    from concourse.bass import AP
    ap = handle[:]
    return AP(ap.tensor, ap.offset + offset, [list(d) for d in dims])


def _emit(tc, nc, io):
    import concourse.mybir as mybir
    from concourse import bass, tile  # noqa: F401
    from concourse.mybir import AluOpType as op, AxisListType as ax

    f32 = mybir.dt.float32
    bf16 = mybir.dt.bfloat16

    (ctr_d, cadd_d, iota16_d, c16m_d, g128_d, iotap_d, attr_d, idxs_d, out_d) = io

    import contextlib
    with contextlib.ExitStack() as ctx:
        cpool = ctx.enter_context(tc.tile_pool(name="consts", bufs=1))
        wpool = ctx.enter_context(tc.tile_pool(name="work", bufs=1))
        spool = ctx.enter_context(tc.tile_pool(name="scratch", bufs=1))
        bpool = ctx.enter_context(tc.tile_pool(name="idxb", bufs=2))
        rpool = ctx.enter_context(tc.tile_pool(name="rbcast", bufs=2))
        r0pool = ctx.enter_context(tc.tile_pool(name="rbcast0", bufs=1))
        opool = ctx.enter_context(tc.tile_pool(name="onehot", bufs=3))
        ppool = ctx.enter_context(tc.tile_pool(name="psum", bufs=3, space="PSUM"))
        fpool = ctx.enter_context(tc.tile_pool(name="outstg", bufs=3))

        def load(pool, src, shape, dtype=f32):
            t = pool.tile(shape, dtype, name=f"ld_{src.name}")
            nc.sync.dma_start(out=t[:], in_=src[:])
            return t

        # split ctr load: slices 0-1 (cols [0, 62)) arrive first so the DVE
        # pipeline starts early; the rest streams in behind it.
        ctr = cpool.tile([P, G * W], f32, name="ld_ctr_win")
        nc.sync.dma_start(out=ctr[:, :62], in_=ctr_d[:][:, :62])
        iota16 = load(cpool, iota16_d, [P, 16])
        i31 = load(cpool, g128_d, [P, 31])
        i31d64 = load(cpool, c16m_d, [P, 31])
        cadd = load(cpool, cadd_d, [P, 1])
        iotap = load(cpool, iotap_d, [P, 1])
        nc.sync.dma_start(out=ctr[:, 62:], in_=ctr_d[:][:, 62:])
        # bf16 casts of the small consts are emitted inside slice 0 (after
        # the first plane op) so they don't block the DVE pipeline start.
        iota16b = cpool.tile([P, 16], bf16, name="iota16b")
        # i31c[p, w] = w + cadd[p]: folds the block-local window base into
        # the position weights, so the pos-reduce emits idxrel directly
        # (sum_w EQ16[r,w] == 1 exactly, so the cadd term survives once).
        i31c = cpool.tile([P, 31], bf16, name="i31c")

        # attr slab: slab[p, 128*m + d] = attr2[core0 + 32*m + p, d] (bf16)
        # loaded in 4 chunks of 16 blocks each for DMA-queue parallelism.
        slab = cpool.tile([P, NBLK * D], bf16, name="slab")
        for h in range(4):
            nc.sync.dma_start(
                out=slab[:, h * 16 * D:(h + 1) * 16 * D],
                in_=_dview(attr_d, (h * 16 * 32) * D,
                           [(D, P), (32 * D, 16), (1, D)]))

        def tt(o, a, b, alu):
            nc.vector.tensor_tensor(out=o, in0=a, in1=b, op=alu)

        def red(o, a, alu=op.add):
            nc.vector.tensor_reduce(out=o, in_=a, axis=ax.X, op=alu)

        _wcnt = [0]

        g0 = 0
        pending = None
        for sl, NG in enumerate(SLICES):

            # ---- rank pipeline (DVE): pos[r] in [0,31) per (q, r) ----
            def wtile(n):
                _wcnt[0] += 1
                return wpool.tile([P, n], f32, name=f"w{_wcnt[0]}")

            cof = g0 * W

            # key[w] = |c_q - c_w| + w/64 : exact f32 (dist<=1e5 int, 17+6
            # bits < 24), unique per window, orders exactly by (dist, index).
            diff = wtile(31 * NG)
            tt(diff, _view(ctr, cof + 15, [(W, NG), (0, 31)]),
                     _view(ctr, cof + 0, [(W, NG), (1, 31)]), op.subtract)
            absd = wtile(31 * NG)
            nc.vector.scalar_tensor_tensor(
                out=absd, in0=diff, scalar=-1.0, in1=diff, op0=op.mult,
                op1=op.max)
            key = wtile(31 * NG)
            tt(key, absd, _view(i31d64, 0, [(0, NG), (1, 31)]), op.add)
            # rank[w] = #{w' : key(w') < key(w)} via one [w, w'] plane
            RK = spool.tile([P, 961 * NG], bf16, name=f"rkpl{sl}", tag=f"plane{sl}")
            tt(RK, _view(key, 0, [(31, NG), (1, 31), (0, 31)]),
                   _view(key, 0, [(31, NG), (0, 31), (1, 31)]), op.is_gt)
            if sl == 0:
                nc.vector.tensor_copy(out=iota16b, in_=iota16)
                nc.vector.tensor_scalar(out=i31c, in0=i31,
                                        scalar1=cadd[:, 0:1], scalar2=None,
                                        op0=op.add)
            rank = wpool.tile([P, 31 * NG], bf16, name=f"rank{sl}", tag=f"rankw{sl}")
            with nc.allow_low_precision(reason="0/1 plane sums <= 31, exact in bf16"):
                red(rank, _view(RK, 0, [(961, NG), (31, 31), (1, 31)]))
            # pos[r] = sum_w [rank(w) == r] * w
            EQ16 = spool.tile([P, 496 * NG], bf16, name=f"eq16{sl}", tag=f"plane2_{sl}")
            tt(EQ16, _view(iota16b, 0, [(0, NG), (1, 16), (0, 31)]),
                     _view(rank, 0, [(31, NG), (0, 16), (1, 31)]), op.is_equal)
            POSP = spool.tile([P, 496 * NG], bf16, name=f"posp{sl}", tag=f"plane3_{sl}")
            tt(POSP, EQ16, _view(i31c, 0, [(0, NG), (0, 16), (1, 31)]), op.mult)
            # block-local neighbor row: idxrel = (q mod 32) + pos in [0, 62)
            idxb = bpool.tile([P, 16 * NG], bf16, name=f"idxb{sl}", tag="idxb")
            with nc.allow_low_precision(reason="one-hot dot, values <= 61 exact in bf16"):
                red(idxb[:], _view(POSP, 0, [(496, NG), (31, 16), (1, 31)]))

            # ---- DRAM round-trip: slot-linear stage, partition-bcast read ----
            # slot = q*16 + r = 2048*g + 16*p + r
            nc.sync.dma_start(
                out=_dview(idxs_d, 2048 * g0, [(16, P), (2048, NG), (1, 16)]),
                in_=idxb[:])
            rp = r0pool if sl == 0 else rpool
            R = rp.tile([P, 2048 * NG], bf16, name=f"R{sl}",
                        tag="R0" if sl == 0 else "R")
            for bi in range(NG):
                nc.sync.dma_start(
                    out=R[:, 2048 * bi:2048 * (bi + 1)],
                    in_=_dview(idxs_d, 2048 * (g0 + bi), [(0, P), (1, 2048)]))

            # ---- per 32-query span: onehot (DVE, 4 spans per op), matmul
            # (PE), PSUM-bank-pair copy (ACT) + store.  Emission of slice
            # k's gather work is delayed until after slice k+1's rank ops:
            # the DVE queue is in-order, so an onehot waiting on its R
            # broadcast DMA would head-of-line block the next slice's ranks.
            def gather_work(sl, g0, NG, R, bis):
                for bi in bis:
                    oh = opool.tile([P, 4 * MM_N], bf16, name=f"oh{sl}_{bi}",
                                    tag="oh")
                    nc.vector.tensor_scalar(
                        out=oh, in0=R[:, 4 * MM_N * bi:4 * MM_N * (bi + 1)],
                        scalar1=iotap[:, 0:1], scalar2=None, op0=op.is_equal)
                    gf = fpool.tile([P, 4 * MM_N], bf16, name=f"gf{sl}_{bi}",
                                    tag="gf")
                    for hf in range(2):
                        ps = ppool.tile([P, 2 * MM_N], f32,
                                        name=f"ps{sl}_{bi}_{hf}", tag="ps")
                        for k in range(2):
                            m = 4 * (g0 + bi) + 2 * hf + k
                            nc.tensor.matmul(
                                ps[:, MM_N * k:MM_N * (k + 1)],
                                slab[:, D * m:D * (m + 1)],
                                oh[:, MM_N * (2 * hf + k):
                                   MM_N * (2 * hf + k + 1)],
                                start=True, stop=True)
                        nc.scalar.copy(
                            out=gf[:, 2 * MM_N * hf:2 * MM_N * (hf + 1)],
                            in_=ps)
                    m0 = 4 * (g0 + bi)
                    # the reserved batch (emitted last) drains in 2 stores
                    last = (sl == 0 and bi == SLICES[0] - 1)
                    for st in range(2 if last else 1):
                        w2 = 2 * MM_N
                        o = st * w2
                        nc.scalar.dma_start(
                            out=_dview(out_d, MM_N * m0 + (o if last else 0),
                                       [(HALF * K, P),
                                        (1, w2 if last else 4 * MM_N)]),
                            in_=gf[:, o:o + w2] if last else gf[:])

            # delayed emission: slice k's gathers run under slice k+1's
            # ranks.  Slice 0's final batch is RESERVED for the very end:
            # its R has been resident for the whole kernel, so the kernel
            # closes with a zero-R-latency gather chain instead of waiting
            # on the last slice's DRAM round-trip.
            if pending is not None:
                psl, pg0, pNG, pR = pending
                bis = list(range(pNG - 1)) if psl == 0 else list(range(pNG))
                gather_work(psl, pg0, pNG, pR, bis)
            pending = (sl, g0, NG, R)
            if sl == 0:
                work0 = pending
            g0 += NG
        gather_work(*pending, list(range(pending[2])))
        gather_work(*work0, [SLICES[0] - 1])


def build():
    """Build + compile the SPMD program once. Returns the Bacc."""
    if "prog" in _CACHE:
        return _CACHE["prog"]
    from concourse import bacc, tile
    import concourse.mybir as mybir

    f32 = mybir.dt.float32
    bf16 = mybir.dt.bfloat16
    nc = bacc.Bacc("TRN2", target_bir_lowering=False, debug=False,
                   num_devices=NCORES)
    ctr_d = nc.declare_dram_parameter("ctr_win", [P, G * W], f32, isOutput=False)
    cadd_d = nc.declare_dram_parameter("c_add", [P, 1], f32, isOutput=False)
    iota16_d = nc.declare_dram_parameter("c_iota16", [P, 16], f32, isOutput=False)
    c16m_d = nc.declare_dram_parameter("c_i31d64", [P, 31], f32, isOutput=False)
    g128_d = nc.declare_dram_parameter("c_i31", [P, 31], f32, isOutput=False)
    iotap_d = nc.declare_dram_parameter("c_iotap", [P, 1], f32, isOutput=False)
    attr_d = nc.declare_dram_parameter("attr_slab", [SLABROWS, D], bf16,
                                       isOutput=False)
    idxs_d = nc.dram_tensor("idx_scratch", [HALF * K], bf16)
    out_d = nc.declare_dram_parameter("out", [D, HALF * K], bf16, isOutput=True)

    io = (ctr_d, cadd_d, iota16_d, c16m_d, g128_d, iotap_d, attr_d, idxs_d,
          out_d)
    with tile.TileContext(nc) as tc:
        _emit(tc, nc, io)
    nc.compile()
    _CACHE["prog"] = nc
    return nc


def host_inputs(first_index, attr):
    """Shard + pad on the host. Returns in_maps (one dict per core)."""
    center = np.asarray(first_index)[..., 0].astype(np.float32)  # [B, L]
    attr = np.ascontiguousarray(np.asarray(attr), dtype=np.float32)

    import ml_dtypes
    attr_bf16 = np.zeros((ROWS2, D), ml_dtypes.bfloat16)
    attr_bf16[FPAD:FPAD + B * L] = attr.reshape(B * L, D).astype(ml_dtypes.bfloat16)
    # per-core slab: local row l <-> original flat row b*L + r0 + l - 15

    LPAD = L + 32
    cpad = np.empty((B, LPAD), np.float32)
    cpad[:, :16] = -BIG
    cpad[:, 16:16 + L] = center
    cpad[:, 16 + L:] = BIG

    p = np.arange(P)
    gg = np.arange(G)
    t = np.arange(W)
    iota16 = np.broadcast_to(np.arange(16, dtype=np.float32), (P, 16)).copy()
    i31 = np.arange(31, dtype=np.float32)
    consts = {
        "c_add": (p % 32).astype(np.float32)[:, None],
        "c_iota16": iota16,
        "c_i31d64": np.broadcast_to(i31 / 64.0, (P, 31)).copy(),
        "c_i31": np.broadcast_to(i31, (P, 31)).copy(),
        "c_iotap": p.astype(np.float32)[:, None],
    }

    in_maps = []
    for c in range(NCORES):
        b, h = divmod(c, 2)
        r0 = h * HALF
        # ctr_win[p, g*31 + t] = cpad[b, r0 + g*128 + p + t + 1]
        idx = r0 + gg[None, :, None] * P + p[:, None, None] + t[None, None, :] + 1
        ctr_win = cpad[b][idx].reshape(P, G * W).astype(np.float32)
        m = dict(consts)
        m["ctr_win"] = np.ascontiguousarray(ctr_win)
        s0 = b * L + r0 + 1
        m["attr_slab"] = np.ascontiguousarray(attr_bf16[s0:s0 + SLABROWS])
        in_maps.append(m)
    return in_maps


def kernel(first_index, attr):
    from concourse.bass_utils import run_bass_kernel_spmd

    nc = build()
    in_maps = host_inputs(first_index, attr)
    res = run_bass_kernel_spmd(nc, in_maps, list(range(NCORES)))
    out = np.empty((B, L, K, D), np.float32)
    for c in range(NCORES):
        b, h = divmod(c, 2)
        r0 = h * HALF
        o = np.asarray(res.results[c]["out"], dtype=np.float32)  # [D, HALF*K]
        out[b, r0:r0 + HALF] = o.T.reshape(HALF, K, D)
    return out
